# revision 17
# baseline (speedup 1.0000x reference)
"""Episodic-memory retrieval (cosine top-5 + softmax-weighted gather) on 8 TRN2 cores.

Strategy (memory-sharded coarse ranking + exact rescore), v2:
  - memory table sharded row-wise across 8 cores (8192 rows each).
  - Phase P: normalize the local mem shard (norms via ones-matmul on PE,
    sharing the M-phase PSUM pool), write bf16 columns to 16 per-tile DRAM
    buffers so phase M can start consuming while P still runs.
  - Phase M: sims = x @ mem_norm.T for all 4096 queries against the local
    shard. Each [128 x 2048] strip accumulates in a 4-bank PSUM tile
    (kc-outer / cti-inner so the stationary operand repeats 4x), then
    hardware top-8 (nc.vector.max / max_index) reads straight from PSUM —
    no PSUM->SBUF copy at all.
  - Phase C: per query block (1024 queries), AllToAll exchanges exactly the
    candidate rows each core needs (128KB/core instead of an 8MB AllGather),
    overlapped under the next block's matmuls.
  - Phase F: per block, each core rescores its interleaved 128-query tile:
    merge 256 candidates -> top-8, gather rows (indirect DMA), exact fp32
    rescore (normalize + dot, like the reference), top-5, softmax, weighted
    sum. Only the last block's F is exposed after the matmuls end.
"""
import numpy as np
import ml_dtypes

import concourse.bacc as bacc
import concourse.bass as bass
import concourse.mybir as mybir
import concourse.tile as tile
from concourse.bass_utils import run_bass_kernel_spmd

F32 = mybir.dt.float32
BF16 = mybir.dt.bfloat16
U32 = mybir.dt.uint32
OP = mybir.AluOpType
ACTF = mybir.ActivationFunctionType

P = 128
K = 5
R = 8                         # rescored candidates per query
NCORES = 8

B, D, C = 4096, 1024, 65536
CL = C // NCORES              # mem rows per core (8192)
NKC = D // P                  # contraction chunks (8)
CT = 512                      # columns per wn DRAM tile / PSUM bank
NCT = CL // CT                # wn tiles per core (16)
QW = 2048                     # strip width (one PSUM strip = 4 banks)
NQUAR = CL // QW              # strips per core (4)
QCT = QW // CT                # col tiles per strip (4)
QBT = 8                       # query tiles per block
NQB = B // (QBT * P)          # query blocks (4)
NCAND = NQUAR * 8             # local candidates per query (32)
MCAND = NCORES * NCAND        # merged candidates per query (256)
QL = NQB * P                  # queries finalized per core (512)

_CACHE = {}


def _build():
    nc = bacc.Bacc("TRN2", target_bir_lowering=False, debug=False,
                   num_devices=NCORES)

    memt = nc.dram_tensor("memt", [D, CL], F32, kind="ExternalInput").ap()
    mems = nc.dram_tensor("mems", [CL, D], F32, kind="ExternalInput").ap()
    xt = nc.dram_tensor("xt", [D, B], BF16, kind="ExternalInput").ap()
    memf = nc.dram_tensor("memf", [C, D], F32, kind="ExternalInput").ap()
    xsl = nc.dram_tensor("xsl", [QL, D], F32, kind="ExternalInput").ap()
    coff = nc.dram_tensor("coff", [1, 1], F32, kind="ExternalInput").ap()
    out = nc.dram_tensor("out", [QL, D], F32, kind="ExternalOutput").ap()

    memt_v = memt.rearrange("(kc p) c -> p kc c", p=P)
    xt_v = xt.rearrange("(kc p) q -> p kc q", p=P)

    with tile.TileContext(nc) as tc:
        with tc.tile_pool(name="const", bufs=1) as pc, \
             tc.tile_pool(name="dram", bufs=1, space="DRAM") as dr, \
             tc.tile_pool(name="psum", bufs=2, space="PSUM") as pps:
            wn_ct = [dr.tile([P, NKC, CT], BF16, name=f"wn_{ct}")
                     for ct in range(NCT)]
            cand_qb = [dr.tile([QBT * P, 2 * NCAND], F32, name=f"cand_{qb}")
                       for qb in range(NQB)]
            cand_x = [dr.tile([QBT * P, 2 * NCAND], F32, name=f"candx_{qb}")
                      for qb in range(NQB)]

            ones_t = pc.tile([P, P], BF16, name="ones_t")
            nc.vector.memset(ones_t[:], 1.0)
            coff_t = pc.tile([1, 1], F32, name="coff_t")
            nc.sync.dma_start(coff_t[:], coff)
            coff_b = pc.tile([P, 1], F32, name="coff_b")
            nc.gpsimd.partition_broadcast(coff_b[:], coff_t[:])
            # per-candidate-column additive offset: quar*QW + core_off
            qoff = pc.tile([P, NCAND], F32, name="qoff")
            for q in range(NQUAR):
                nc.vector.memset(qoff[:, q * 8:(q + 1) * 8], float(q * QW))
            nc.vector.tensor_scalar(out=qoff[:], in0=qoff[:],
                                    scalar1=coff_b[:, 0:1], scalar2=None,
                                    op0=OP.add)

            # ---------------- Phase P: normalize mem shard -> wn (bf16) -----
            # PE-free: row-major squares-sum on DVE, reciprocal broadcast via
            # a tiny DRAM transpose bounce, column scaling on GpSimd. Keeps
            # the PE instruction queue 100% matmul so phase M starts cold.
            with tc.tile_pool(name="pp", bufs=2) as pp, \
                 tc.tile_pool(name="prow", bufs=4) as prow, \
                 tc.tile_pool(name="pnrm", bufs=3) as pnrm:
                invd = [dr.tile([1, 4 * P], F32, name=f"invd_{ct}")
                        for ct in range(NCT)]
                for ct in range(NCT):
                    cs = slice(ct * CT, (ct + 1) * CT)
                    nsq = pnrm.tile([P, 4], F32, tag="nsq")
                    for rt in range(4):
                        r0 = ct * CT + rt * P
                        rowt = prow.tile([P, D], F32, tag="rowt")
                        nc.sync.dma_start(rowt[:], mems[r0:r0 + P, :])
                        scrp = prow.tile([P, D], BF16, tag="scrp",
                                         name=f"scrp_{ct}_{rt}")
                        # ACT LUT square: approximate, but these norms feed
                        # only the coarse ranking (exact rescore renorms).
                        nc.scalar.activation(scrp[:], rowt[:], ACTF.Square,
                                             accum_out=nsq[:, rt:rt + 1])
                    nst = pnrm.tile([P, 4], F32, tag="nst")
                    nc.scalar.activation(nst[:], nsq[:], ACTF.Sqrt)
                    nrc = pnrm.tile([P, 4], F32, tag="nrc")
                    nc.vector.reciprocal(nrc[:], nst[:])
                    # SBUF [p, r] -> DRAM flat[r*128+p] (transpose bounce)
                    nc.sync.dma_start(
                        invd[ct].rearrange("o (r p) -> (o p) r", p=P),
                        nrc[:])
                    inv_row = pnrm.tile([1, CT], F32, tag="inv_row")
                    nc.sync.dma_start(inv_row[:], invd[ct][:])
                    inv_bc = pnrm.tile([P, CT], F32, tag="inv_bc")
                    nc.gpsimd.partition_broadcast(inv_bc[:], inv_row[:])
                    mslab = pp.tile([P, NKC, CT], F32, tag="mslab")
                    nc.sync.dma_start(mslab[:], memt_v[:, :, cs])
                    wnt = pp.tile([P, NKC, CT], BF16, tag="wnt")
                    for kc in range(NKC):
                        nc.vector.tensor_tensor(out=wnt[:, kc, :],
                                                in0=mslab[:, kc, :],
                                                in1=inv_bc[:], op=OP.mult)
                    nc.sync.dma_start(wn_ct[ct][:], wnt[:])

            # ---------------- Phase M + C + F, pipelined per query block ----
            with tc.tile_pool(name="px", bufs=2) as px, \
                 tc.tile_pool(name="pg", bufs=1) as pg, \
                 tc.tile_pool(name="pw", bufs=2) as pw, \
                 tc.tile_pool(name="pcand", bufs=2 * QBT) as pcand, \
                 tc.tile_pool(name="pf", bufs=2) as pf:
                def emit_f(qb):
                    """Merge + exact rescore for block qb's 128-query tile.

                    Emitted one quar into the NEXT block so the in-order DVE
                    queue never head-of-line blocks on AllToAll latency.
                    """
                    ctile = pf.tile([P, NCORES, 2 * NCAND], F32, tag="ctile",
                                    name=f"ctile_{qb}")
                    for cc in range(NCORES):
                        nc.sync.dma_start(
                            ctile[:, cc, :],
                            cand_x[qb][cc * P:(cc + 1) * P, :])
                    cvp = pf.tile([P, MCAND], F32, tag="cvp")
                    nc.vector.tensor_copy(cvp[:], ctile[:, :, 0:NCAND])
                    cip1 = pf.tile([P, MCAND], F32, tag="cip1")
                    nc.vector.tensor_scalar(out=cip1[:],
                                            in0=ctile[:, :, NCAND:2 * NCAND],
                                            scalar1=1.0, scalar2=None,
                                            op0=OP.add)
                    m8 = pf.tile([P, 8], F32, tag="m8")
                    nc.vector.max(out=m8[:], in_=cvp[:])
                    gfx = pf.tile([P, 8], F32, tag="gfx")
                    giu = pf.tile([P, 8], U32, tag="giu")
                    g = pg.tile([P, R, D], F32, tag="g", name=f"g_{qb}")
                    for i in range(R):
                        sel = pf.tile([P, MCAND], F32, tag="sel")
                        nc.vector.scalar_tensor_tensor(
                            out=sel[:], in0=cvp[:], scalar=m8[:, i:i + 1],
                            in1=cip1[:], op0=OP.is_equal, op1=OP.mult)
                        red = pf.tile([P, 1], F32, tag="red")
                        nc.vector.tensor_reduce(out=red[:], in_=sel[:],
                                                axis=mybir.AxisListType.X,
                                                op=OP.max)
                        nc.vector.tensor_scalar(out=gfx[:, i:i + 1],
                                                in0=red[:], scalar1=-1.0,
                                                scalar2=None, op0=OP.add)
                        nc.vector.tensor_copy(giu[:, i:i + 1],
                                              gfx[:, i:i + 1])
                        nc.gpsimd.indirect_dma_start(
                            out=g[:, i, :], out_offset=None, in_=memf,
                            in_offset=bass.IndirectOffsetOnAxis(
                                ap=giu[:, i:i + 1], axis=0))
                    xrow = pf.tile([P, D], F32, tag="xrow")
                    nc.sync.dma_start(xrow[:], xsl[qb * P:(qb + 1) * P, :])
                    scratch = pf.tile([P, D], F32, tag="scratch")
                    xsq = pf.tile([P, 1], F32, tag="xsq")
                    nc.vector.scalar_tensor_tensor(
                        out=scratch[:], in0=xrow[:], scalar=1.0, in1=xrow[:],
                        op0=OP.mult, op1=OP.mult, accum_out=xsq[:])
                    xnm = pf.tile([P, 1], F32, tag="xnm")
                    nc.scalar.activation(xnm[:], xsq[:], ACTF.Sqrt)
                    xrcp = pf.tile([P, 1], F32, tag="xrcp")
                    nc.vector.reciprocal(xrcp[:], xnm[:])
                    xrn = pf.tile([P, D], F32, tag="xrn")
                    nc.vector.tensor_scalar(out=xrn[:], in0=xrow[:],
                                            scalar1=xrcp[:, 0:1], scalar2=None,
                                            op0=OP.mult)
                    msq = pf.tile([P, R], F32, tag="msq")
                    for i in range(R):
                        scr_i = pf.tile([P, D], F32, tag="scratch",
                                        name=f"scr_{qb}_{i}")
                        nc.vector.scalar_tensor_tensor(
                            out=scr_i[:], in0=g[:, i, :], scalar=1.0,
                            in1=g[:, i, :], op0=OP.mult, op1=OP.mult,
                            accum_out=msq[:, i:i + 1])
                    mnm = pf.tile([P, R], F32, tag="mnm")
                    nc.scalar.activation(mnm[:], msq[:], ACTF.Sqrt)
                    mrcp = pf.tile([P, R], F32, tag="mrcp")
                    nc.vector.reciprocal(mrcp[:], mnm[:])
                    d8 = pf.tile([P, R], F32, tag="d8")
                    for i in range(R):
                        # (g_i * (1/||m_i||)) * x_hat, summed: exact fp32 dot
                        scr_d = pf.tile([P, D], F32, tag="scratch",
                                        name=f"scrd_{qb}_{i}")
                        nc.vector.scalar_tensor_tensor(
                            out=scr_d[:], in0=g[:, i, :],
                            scalar=mrcp[:, i:i + 1], in1=xrn[:],
                            op0=OP.mult, op1=OP.mult,
                            accum_out=d8[:, i:i + 1])
                    s8 = pf.tile([P, R], F32, tag="s8")
                    nc.vector.max(out=s8[:], in_=d8[:])
                    mask = pf.tile([P, R], F32, tag="mask")
                    nc.vector.tensor_scalar(out=mask[:], in0=d8[:],
                                            scalar1=s8[:, K - 1:K],
                                            scalar2=None, op0=OP.is_ge)
                    e8 = pf.tile([P, R], F32, tag="e8")
                    nc.vector.tensor_scalar(out=e8[:], in0=d8[:],
                                            scalar1=s8[:, 0:1], scalar2=None,
                                            op0=OP.subtract)
                    nc.scalar.activation(e8[:], e8[:], ACTF.Exp)
                    nc.vector.tensor_tensor(out=e8[:], in0=e8[:], in1=mask[:],
                                            op=OP.mult)
                    esum = pf.tile([P, 1], F32, tag="esum")
                    nc.vector.tensor_reduce(out=esum[:], in_=e8[:],
                                            axis=mybir.AxisListType.X,
                                            op=OP.add)
                    rs = pf.tile([P, 1], F32, tag="rs")
                    nc.vector.reciprocal(rs[:], esum[:])
                    w8 = pf.tile([P, R], F32, tag="w8")
                    nc.vector.tensor_scalar(out=w8[:], in0=e8[:],
                                            scalar1=rs[:, 0:1], scalar2=None,
                                            op0=OP.mult)
                    acc = pg.tile([P, D], F32, tag="acc", name=f"acc_{qb}")
                    nc.vector.tensor_scalar(out=acc[:], in0=g[:, 0, :],
                                            scalar1=w8[:, 0:1], scalar2=None,
                                            op0=OP.mult)
                    for i in range(1, R):
                        nc.vector.scalar_tensor_tensor(
                            out=acc[:], in0=g[:, i, :],
                            scalar=w8[:, i:i + 1], in1=acc[:],
                            op0=OP.mult, op1=OP.add)
                    nc.sync.dma_start(out[qb * P:(qb + 1) * P, :], acc[:])

                for qb in range(NQB):
                    qs = slice(qb * QBT * P, (qb + 1) * QBT * P)
                    xq = px.tile([P, NKC, QBT * P], BF16, tag="xq")
                    nc.sync.dma_start(xq[:], xt_v[:, :, qs])
                    pk = [pcand.tile([P, 2 * NCAND], F32, tag="pk",
                                     name=f"pk_{qb}_{qt}") for qt in range(QBT)]
                    ci = [pcand.tile([P, NCAND], U32, tag="ci",
                                     name=f"ci_{qb}_{qt}") for qt in range(QBT)]
                    for quar in range(NQUAR):
                        ws = pw.tile([P, NKC, QW], BF16, tag="ws")
                        for cti in range(QCT):
                            ct = quar * QCT + cti
                            nc.sync.dma_start(
                                ws[:, :, cti * CT:(cti + 1) * CT],
                                wn_ct[ct][:])
                        for qt in range(QBT):
                            pstrip = pps.tile([P, QW], F32, tag="pstrip",
                                              name=f"ps_{qb}_{quar}_{qt}")
                            for kc in range(NKC):
                                lhs = xq[:, kc, qt * P:(qt + 1) * P]
                                for cti in range(QCT):
                                    nc.tensor.matmul(
                                        out=pstrip[:, cti * CT:(cti + 1) * CT],
                                        lhsT=lhs,
                                        rhs=ws[:, kc, cti * CT:(cti + 1) * CT],
                                        start=(kc == 0), stop=(kc == NKC - 1))
                            q8 = slice(quar * 8, (quar + 1) * 8)
                            nc.vector.max(out=pk[qt][:, q8], in_=pstrip[:])
                            nc.vector.max_index(out=ci[qt][:, q8],
                                                in_max=pk[qt][:, q8],
                                                in_values=pstrip[:])
                        if quar == 1 and qb > 0:
                            emit_f(qb - 1)
                    for qt in range(QBT):
                        ix = slice(NCAND, 2 * NCAND)
                        nc.vector.tensor_copy(pk[qt][:, ix], ci[qt][:])
                        nc.vector.tensor_tensor(out=pk[qt][:, ix],
                                                in0=pk[qt][:, ix],
                                                in1=qoff[:], op=OP.add)
                        row = qt * P
                        nc.sync.dma_start(cand_qb[qb][row:row + P, :],
                                          pk[qt][:, :])

                    # ---- Phase C: exchange candidates for this block ------
                    nc.gpsimd.collective_compute(
                        "AllToAll", OP.bypass,
                        replica_groups=[list(range(NCORES))],
                        ins=[cand_qb[qb][:]], outs=[cand_x[qb][:]])
                emit_f(NQB - 1)

    nc.compile()
    return nc


def _in_maps(x, memory):
    xtb = np.ascontiguousarray(x.T).astype(ml_dtypes.bfloat16)
    maps = []
    for j in range(NCORES):
        mems_j = np.ascontiguousarray(memory[j * CL:(j + 1) * CL])
        memt_j = np.ascontiguousarray(mems_j.T)
        # core j finalizes query tile j of every block: rows qb*1024 + j*128
        rows = np.concatenate([
            np.arange(qb * QBT * P + j * P, qb * QBT * P + (j + 1) * P)
            for qb in range(NQB)])
        maps.append(dict(
            memt=memt_j, mems=mems_j, xt=xtb, memf=memory,
            xsl=np.ascontiguousarray(x[rows]),
            coff=np.full((1, 1), float(j * CL), dtype=np.float32)))
    return maps


def run(x, memory, trace=False, trace_cores=None):
    if "nc" not in _CACHE:
        _CACHE["nc"] = _build()
    nc = _CACHE["nc"]
    res = run_bass_kernel_spmd(nc, _in_maps(x, memory),
                               list(range(NCORES)),
                               trace=trace, trace_cores=trace_cores)
    outp = np.empty((B, D), dtype=np.float32)
    for j in range(NCORES):
        for qb in range(NQB):
            outp[qb * QBT * P + j * P: qb * QBT * P + (j + 1) * P] = \
                res.results[j]["out"][qb * P:(qb + 1) * P]
    return outp, res


def kernel(x, memory, k):
    assert int(k) == K
    x = np.asarray(x, dtype=np.float32)
    memory = np.asarray(memory, dtype=np.float32)
    outp, _ = run(x, memory)
    return outp


# revision 19
# speedup vs baseline: 1.2106x; 1.2106x over previous
"""Episodic-memory retrieval (cosine top-5 + softmax-weighted gather) on 8 TRN2 cores.

Strategy (memory-sharded coarse ranking + exact rescore), v2:
  - memory table sharded row-wise across 8 cores (8192 rows each).
  - Phase P: normalize the local mem shard (norms via ones-matmul on PE,
    sharing the M-phase PSUM pool), write bf16 columns to 16 per-tile DRAM
    buffers so phase M can start consuming while P still runs.
  - Phase M: sims = x @ mem_norm.T for all 4096 queries against the local
    shard. Each [128 x 2048] strip accumulates in a 4-bank PSUM tile
    (kc-outer / cti-inner so the stationary operand repeats 4x), then
    hardware top-8 (nc.vector.max / max_index) reads straight from PSUM —
    no PSUM->SBUF copy at all.
  - Phase C: per query block (1024 queries), AllToAll exchanges exactly the
    candidate rows each core needs (128KB/core instead of an 8MB AllGather),
    overlapped under the next block's matmuls.
  - Phase F: per block, each core rescores its interleaved 128-query tile:
    merge 256 candidates -> top-8, gather rows (indirect DMA), exact fp32
    rescore (normalize + dot, like the reference), top-5, softmax, weighted
    sum. Only the last block's F is exposed after the matmuls end.
"""
import numpy as np
import ml_dtypes

import concourse.bacc as bacc
import concourse.bass as bass
import concourse.mybir as mybir
import concourse.tile as tile
from concourse.bass_utils import run_bass_kernel_spmd

F32 = mybir.dt.float32
BF16 = mybir.dt.bfloat16
U32 = mybir.dt.uint32
OP = mybir.AluOpType
ACTF = mybir.ActivationFunctionType

P = 128
K = 5
R = 8                         # rescored candidates per query
NCORES = 8

B, D, C = 4096, 1024, 65536
CL = C // NCORES              # mem rows per core (8192)
NKC = D // P                  # contraction chunks (8)
CT = 512                      # columns per wn DRAM tile / PSUM bank
NCT = CL // CT                # wn tiles per core (16)
QW = 2048                     # strip width (one PSUM strip = 4 banks)
NQUAR = CL // QW              # strips per core (4)
QCT = QW // CT                # col tiles per strip (4)
QBT = 8                       # query tiles per block
NQB = B // (QBT * P)          # query blocks (4)
NCAND = NQUAR * 8             # local candidates per query (32)
MCAND = NCORES * NCAND        # merged candidates per query (256)
QL = NQB * P                  # queries finalized per core (512)

_CACHE = {}


def _build():
    nc = bacc.Bacc("TRN2", target_bir_lowering=False, debug=False,
                   num_devices=NCORES)

    memt = nc.dram_tensor("memt", [D, CL], F32, kind="ExternalInput").ap()
    xt = nc.dram_tensor("xt", [D, B], BF16, kind="ExternalInput").ap()
    memf = nc.dram_tensor("memf", [C, D], F32, kind="ExternalInput").ap()
    xsl = nc.dram_tensor("xsl", [QL, D], F32, kind="ExternalInput").ap()
    coff = nc.dram_tensor("coff", [1, 1], F32, kind="ExternalInput").ap()
    out = nc.dram_tensor("out", [QL, D], F32, kind="ExternalOutput").ap()

    memt_v = memt.rearrange("(kc p) c -> p kc c", p=P)
    xt_v = xt.rearrange("(kc p) q -> p kc q", p=P)

    with tile.TileContext(nc) as tc:
        with tc.tile_pool(name="const", bufs=1) as pc, \
             tc.tile_pool(name="dram", bufs=1, space="DRAM") as dr, \
             tc.tile_pool(name="psum", bufs=2, space="PSUM") as pps:
            wn_ct = [dr.tile([P, NKC, CT], BF16, name=f"wn_{ct}")
                     for ct in range(NCT)]
            cand_qb = [dr.tile([QBT * P, 2 * NCAND], F32, name=f"cand_{qb}")
                       for qb in range(NQB)]
            cand_x = [dr.tile([QBT * P, 2 * NCAND], F32, name=f"candx_{qb}")
                      for qb in range(NQB)]

            ones_t = pc.tile([P, P], BF16, name="ones_t")
            nc.vector.memset(ones_t[:], 1.0)
            coff_t = pc.tile([1, 1], F32, name="coff_t")
            nc.sync.dma_start(coff_t[:], coff)
            coff_b = pc.tile([P, 1], F32, name="coff_b")
            nc.gpsimd.partition_broadcast(coff_b[:], coff_t[:])
            # per-candidate-column additive offset: quar*QW + core_off
            qoff = pc.tile([P, NCAND], F32, name="qoff")
            for q in range(NQUAR):
                nc.vector.memset(qoff[:, q * 8:(q + 1) * 8], float(q * QW))
            nc.vector.tensor_scalar(out=qoff[:], in0=qoff[:],
                                    scalar1=coff_b[:, 0:1], scalar2=None,
                                    op0=OP.add)

            # ---------------- Phase P: normalize mem shard -> wn (bf16) -----
            # Column norms via ones-matmul on the PE; the nps tile shares the
            # M-phase PSUM tag so P and M pipeline through the same 2 buffers.
            with tc.tile_pool(name="pp", bufs=2) as pp, \
                 tc.tile_pool(name="ppsq", bufs=3) as ppsq:
                for ct in range(NCT):
                    cs = slice(ct * CT, (ct + 1) * CT)
                    mslab = pp.tile([P, NKC, CT], F32, tag="mslab")
                    nc.sync.dma_start(mslab[:], memt_v[:, :, cs])
                    npt = pps.tile([P, QW], F32, tag="pstrip",
                                   name=f"nps_{ct}")
                    nps = npt[:, 0:CT]
                    for kc in range(NKC):
                        sq = ppsq.tile([P, CT], BF16, tag="sq")
                        nc.scalar.square(sq[:], mslab[:, kc, :])
                        nc.tensor.matmul(out=nps, lhsT=ones_t[:], rhs=sq[:],
                                         start=(kc == 0), stop=(kc == NKC - 1))
                    std = ppsq.tile([P, CT], F32, tag="std")
                    nc.scalar.activation(std[:], nps, ACTF.Sqrt)
                    inv = ppsq.tile([P, CT], F32, tag="inv")
                    nc.vector.reciprocal(inv[:], std[:])
                    wnt = pp.tile([P, NKC, CT], BF16, tag="wnt")
                    for kc in range(NKC):
                        nc.vector.tensor_tensor(out=wnt[:, kc, :],
                                                in0=mslab[:, kc, :],
                                                in1=inv[:], op=OP.mult)
                    nc.sync.dma_start(wn_ct[ct][:], wnt[:])

            # ---------------- Phase M + C + F, pipelined per query block ----
            with tc.tile_pool(name="px", bufs=2) as px, \
                 tc.tile_pool(name="pg", bufs=1) as pg, \
                 tc.tile_pool(name="pw", bufs=2) as pw, \
                 tc.tile_pool(name="pcand", bufs=2 * QBT) as pcand, \
                 tc.tile_pool(name="pf", bufs=2) as pf:
                def emit_f(qb):
                    """Merge + exact rescore for block qb's 128-query tile.

                    Emitted one quar into the NEXT block so the in-order DVE
                    queue never head-of-line blocks on AllToAll latency.
                    """
                    ctile = pf.tile([P, NCORES, 2 * NCAND], F32, tag="ctile",
                                    name=f"ctile_{qb}")
                    for cc in range(NCORES):
                        nc.sync.dma_start(
                            ctile[:, cc, :],
                            cand_x[qb][cc * P:(cc + 1) * P, :])
                    cvp = pf.tile([P, MCAND], F32, tag="cvp")
                    nc.vector.tensor_copy(cvp[:], ctile[:, :, 0:NCAND])
                    cip1 = pf.tile([P, MCAND], F32, tag="cip1")
                    nc.vector.tensor_scalar(out=cip1[:],
                                            in0=ctile[:, :, NCAND:2 * NCAND],
                                            scalar1=1.0, scalar2=None,
                                            op0=OP.add)
                    m8 = pf.tile([P, 8], F32, tag="m8")
                    nc.vector.max(out=m8[:], in_=cvp[:])
                    gfx = pf.tile([P, 8], F32, tag="gfx")
                    giu = pf.tile([P, 8], U32, tag="giu")
                    g = pg.tile([P, R, D], F32, tag="g", name=f"g_{qb}")
                    for i in range(R):
                        sel = pf.tile([P, MCAND], F32, tag="sel")
                        nc.vector.scalar_tensor_tensor(
                            out=sel[:], in0=cvp[:], scalar=m8[:, i:i + 1],
                            in1=cip1[:], op0=OP.is_equal, op1=OP.mult)
                        red = pf.tile([P, 1], F32, tag="red")
                        nc.vector.tensor_reduce(out=red[:], in_=sel[:],
                                                axis=mybir.AxisListType.X,
                                                op=OP.max)
                        nc.vector.tensor_scalar(out=gfx[:, i:i + 1],
                                                in0=red[:], scalar1=-1.0,
                                                scalar2=None, op0=OP.add)
                        nc.vector.tensor_copy(giu[:, i:i + 1],
                                              gfx[:, i:i + 1])
                        nc.gpsimd.indirect_dma_start(
                            out=g[:, i, :], out_offset=None, in_=memf,
                            in_offset=bass.IndirectOffsetOnAxis(
                                ap=giu[:, i:i + 1], axis=0))
                    xrow = pf.tile([P, D], F32, tag="xrow")
                    nc.sync.dma_start(xrow[:], xsl[qb * P:(qb + 1) * P, :])
                    scratch = pf.tile([P, D], F32, tag="scratch")
                    xsq = pf.tile([P, 1], F32, tag="xsq")
                    nc.vector.scalar_tensor_tensor(
                        out=scratch[:], in0=xrow[:], scalar=1.0, in1=xrow[:],
                        op0=OP.mult, op1=OP.mult, accum_out=xsq[:])
                    xnm = pf.tile([P, 1], F32, tag="xnm")
                    nc.scalar.activation(xnm[:], xsq[:], ACTF.Sqrt)
                    xrcp = pf.tile([P, 1], F32, tag="xrcp")
                    nc.vector.reciprocal(xrcp[:], xnm[:])
                    xrn = pf.tile([P, D], F32, tag="xrn")
                    nc.vector.tensor_scalar(out=xrn[:], in0=xrow[:],
                                            scalar1=xrcp[:, 0:1], scalar2=None,
                                            op0=OP.mult)
                    msq = pf.tile([P, R], F32, tag="msq")
                    for i in range(R):
                        scr_i = pf.tile([P, D], F32, tag="scratch",
                                        name=f"scr_{qb}_{i}")
                        nc.vector.scalar_tensor_tensor(
                            out=scr_i[:], in0=g[:, i, :], scalar=1.0,
                            in1=g[:, i, :], op0=OP.mult, op1=OP.mult,
                            accum_out=msq[:, i:i + 1])
                    mnm = pf.tile([P, R], F32, tag="mnm")
                    nc.scalar.activation(mnm[:], msq[:], ACTF.Sqrt)
                    mrcp = pf.tile([P, R], F32, tag="mrcp")
                    nc.vector.reciprocal(mrcp[:], mnm[:])
                    d8 = pf.tile([P, R], F32, tag="d8")
                    for i in range(R):
                        # (g_i * (1/||m_i||)) * x_hat, summed: exact fp32 dot
                        scr_d = pf.tile([P, D], F32, tag="scratch",
                                        name=f"scrd_{qb}_{i}")
                        nc.vector.scalar_tensor_tensor(
                            out=scr_d[:], in0=g[:, i, :],
                            scalar=mrcp[:, i:i + 1], in1=xrn[:],
                            op0=OP.mult, op1=OP.mult,
                            accum_out=d8[:, i:i + 1])
                    s8 = pf.tile([P, R], F32, tag="s8")
                    nc.vector.max(out=s8[:], in_=d8[:])
                    mask = pf.tile([P, R], F32, tag="mask")
                    nc.vector.tensor_scalar(out=mask[:], in0=d8[:],
                                            scalar1=s8[:, K - 1:K],
                                            scalar2=None, op0=OP.is_ge)
                    e8 = pf.tile([P, R], F32, tag="e8")
                    nc.vector.tensor_scalar(out=e8[:], in0=d8[:],
                                            scalar1=s8[:, 0:1], scalar2=None,
                                            op0=OP.subtract)
                    nc.scalar.activation(e8[:], e8[:], ACTF.Exp)
                    nc.vector.tensor_tensor(out=e8[:], in0=e8[:], in1=mask[:],
                                            op=OP.mult)
                    esum = pf.tile([P, 1], F32, tag="esum")
                    nc.vector.tensor_reduce(out=esum[:], in_=e8[:],
                                            axis=mybir.AxisListType.X,
                                            op=OP.add)
                    rs = pf.tile([P, 1], F32, tag="rs")
                    nc.vector.reciprocal(rs[:], esum[:])
                    w8 = pf.tile([P, R], F32, tag="w8")
                    nc.vector.tensor_scalar(out=w8[:], in0=e8[:],
                                            scalar1=rs[:, 0:1], scalar2=None,
                                            op0=OP.mult)
                    acc = pg.tile([P, D], F32, tag="acc", name=f"acc_{qb}")
                    nc.vector.tensor_scalar(out=acc[:], in0=g[:, 0, :],
                                            scalar1=w8[:, 0:1], scalar2=None,
                                            op0=OP.mult)
                    for i in range(1, R):
                        nc.vector.scalar_tensor_tensor(
                            out=acc[:], in0=g[:, i, :],
                            scalar=w8[:, i:i + 1], in1=acc[:],
                            op0=OP.mult, op1=OP.add)
                    nc.sync.dma_start(out[qb * P:(qb + 1) * P, :], acc[:])

                for qb in range(NQB):
                    qs = slice(qb * QBT * P, (qb + 1) * QBT * P)
                    xq = px.tile([P, NKC, QBT * P], BF16, tag="xq")
                    nc.sync.dma_start(xq[:], xt_v[:, :, qs])
                    pk = [pcand.tile([P, 2 * NCAND], F32, tag="pk",
                                     name=f"pk_{qb}_{qt}") for qt in range(QBT)]
                    ci = [pcand.tile([P, NCAND], U32, tag="ci",
                                     name=f"ci_{qb}_{qt}") for qt in range(QBT)]
                    for quar in range(NQUAR):
                        ws = pw.tile([P, NKC, QW], BF16, tag="ws")
                        for cti in range(QCT):
                            ct = quar * QCT + cti
                            nc.sync.dma_start(
                                ws[:, :, cti * CT:(cti + 1) * CT],
                                wn_ct[ct][:])
                        for qt in range(QBT):
                            pstrip = pps.tile([P, QW], F32, tag="pstrip",
                                              name=f"ps_{qb}_{quar}_{qt}")
                            for kc in range(NKC):
                                lhs = xq[:, kc, qt * P:(qt + 1) * P]
                                for cti in range(QCT):
                                    nc.tensor.matmul(
                                        out=pstrip[:, cti * CT:(cti + 1) * CT],
                                        lhsT=lhs,
                                        rhs=ws[:, kc, cti * CT:(cti + 1) * CT],
                                        start=(kc == 0), stop=(kc == NKC - 1))
                            q8 = slice(quar * 8, (quar + 1) * 8)
                            nc.vector.max(out=pk[qt][:, q8], in_=pstrip[:])
                            nc.vector.max_index(out=ci[qt][:, q8],
                                                in_max=pk[qt][:, q8],
                                                in_values=pstrip[:])
                        if quar == 1 and qb > 0:
                            emit_f(qb - 1)
                    for qt in range(QBT):
                        ix = slice(NCAND, 2 * NCAND)
                        nc.vector.tensor_copy(pk[qt][:, ix], ci[qt][:])
                        nc.vector.tensor_tensor(out=pk[qt][:, ix],
                                                in0=pk[qt][:, ix],
                                                in1=qoff[:], op=OP.add)
                        row = qt * P
                        nc.sync.dma_start(cand_qb[qb][row:row + P, :],
                                          pk[qt][:, :])

                    # ---- Phase C: exchange candidates for this block ------
                    nc.gpsimd.collective_compute(
                        "AllToAll", OP.bypass,
                        replica_groups=[list(range(NCORES))],
                        ins=[cand_qb[qb][:]], outs=[cand_x[qb][:]])
                emit_f(NQB - 1)

    nc.compile()
    return nc


def _in_maps(x, memory):
    xtb = np.ascontiguousarray(x.T).astype(ml_dtypes.bfloat16)
    maps = []
    for j in range(NCORES):
        memt_j = np.ascontiguousarray(memory[j * CL:(j + 1) * CL].T)
        # core j finalizes query tile j of every block: rows qb*1024 + j*128
        rows = np.concatenate([
            np.arange(qb * QBT * P + j * P, qb * QBT * P + (j + 1) * P)
            for qb in range(NQB)])
        maps.append(dict(
            memt=memt_j, xt=xtb, memf=memory,
            xsl=np.ascontiguousarray(x[rows]),
            coff=np.full((1, 1), float(j * CL), dtype=np.float32)))
    return maps


def run(x, memory, trace=False, trace_cores=None):
    if "nc" not in _CACHE:
        _CACHE["nc"] = _build()
    nc = _CACHE["nc"]
    res = run_bass_kernel_spmd(nc, _in_maps(x, memory),
                               list(range(NCORES)),
                               trace=trace, trace_cores=trace_cores)
    outp = np.empty((B, D), dtype=np.float32)
    for j in range(NCORES):
        for qb in range(NQB):
            outp[qb * QBT * P + j * P: qb * QBT * P + (j + 1) * P] = \
                res.results[j]["out"][qb * P:(qb + 1) * P]
    return outp, res


def kernel(x, memory, k):
    assert int(k) == K
    x = np.asarray(x, dtype=np.float32)
    memory = np.asarray(memory, dtype=np.float32)
    outp, _ = run(x, memory)
    return outp
